# revision 1
# baseline (speedup 1.0000x reference)
"""Trainium2 Bass kernel: multi-head elementwise-attention GNN message passing.

Full inputs -> full output. Internally: edges partitioned by destination-node
block across 8 NeuronCores; k/v projections replicated; per-edge gathers via
indirect DMA; segment sums via one-hot matmuls accumulated in PSUM.
"""
import sys
sys.path.insert(0, '/opt/trn_rl_repo')
import math
import numpy as np
import ml_dtypes

import concourse.bass as bass
import concourse.bacc as bacc
import concourse.mybir as mybir
import concourse.tile as tile
from concourse import bass2jax

P = 128
D = 128
N_CORES = 8
KV_BF16 = True

_cache = {}


def _build(nblk_core, t_b, n_all_blk, kv_bf16=KV_BF16, repeat=1, rep_scope='all', seq_gather=False, ablate=''):
    """Build+compile the per-core Bass module.

    nblk_core: node blocks owned by each core (output range)
    t_b:       tiles (128 edges each) per block, fixed
    n_all_blk: total node blocks (padded N / 128), projections replicated
    """
    key = (nblk_core, t_b, n_all_blk, kv_bf16, repeat, rep_scope, seq_gather, ablate)
    if key in _cache:
        return _cache[key]
    n_pad = n_all_blk * P
    n_core = nblk_core * P
    ncols = nblk_core * t_b
    kv_dt = mybir.dt.bfloat16 if kv_bf16 else mybir.dt.float32
    f32 = mybir.dt.float32

    nc = bacc.Bacc("TRN2", target_bir_lowering=False, debug=False,
                   num_devices=N_CORES)
    # ---- I/O ----
    xT = nc.dram_tensor("xT", [P, n_pad], f32, kind="ExternalInput")
    xTq = nc.dram_tensor("xTq", [P, n_core], f32, kind="ExternalInput")
    wk = nc.dram_tensor("wk", [D, D], f32, kind="ExternalInput")
    wv = nc.dram_tensor("wv", [D, D], f32, kind="ExternalInput")
    wq = nc.dram_tensor("wq", [D, D], f32, kind="ExternalInput")
    wo = nc.dram_tensor("wo", [D, D], f32, kind="ExternalInput")
    bkv = nc.dram_tensor("bkv", [P, 2 * D], f32, kind="ExternalInput")
    bq = nc.dram_tensor("bq", [P, D], f32, kind="ExternalInput")
    iotaF3 = nc.dram_tensor("iotaF3", [P, 3 * P], f32, kind="ExternalInput")
    iotaP = nc.dram_tensor("iotaP", [P, 1], f32, kind="ExternalInput")
    ones1 = nc.dram_tensor("ones1", [1, P], f32, kind="ExternalInput")
    srcoff = nc.dram_tensor("srcoff", [P, ncols], mybir.dt.int32,
                            kind="ExternalInput")
    offc = nc.dram_tensor("offc", [P, ncols], f32, kind="ExternalInput")
    offr = nc.dram_tensor("offr", [1, ncols * P], f32, kind="ExternalInput")
    outT = nc.dram_tensor("outT", [P, n_core], f32, kind="ExternalOutput")

    with tile.TileContext(nc) as tc:
        with tc.tile_pool(name="const", bufs=1) as cp, \
             tc.tile_pool(name="qres", bufs=1) as qp, \
             tc.tile_pool(name="dram", bufs=1, space="DRAM") as dp, \
             tc.tile_pool(name="xld", bufs=4) as xp, \
             tc.tile_pool(name="kvw", bufs=4) as kp, \
             tc.tile_pool(name="meta", bufs=3) as mp, \
             tc.tile_pool(name="gath", bufs=12) as gp, \
             tc.tile_pool(name="work", bufs=4) as wp, \
             tc.tile_pool(name="epi", bufs=3) as ep, \
             tc.tile_pool(name="ps", bufs=6, space="PSUM") as pp, \
             tc.tile_pool(name="psz", bufs=1, space="PSUM") as pz:

            # ---- constants to SBUF ----
            wk_s = cp.tile([D, D], f32); nc.sync.dma_start(out=wk_s[:], in_=wk.ap())
            wv_s = cp.tile([D, D], f32); nc.sync.dma_start(out=wv_s[:], in_=wv.ap())
            wq_s = cp.tile([D, D], f32); nc.sync.dma_start(out=wq_s[:], in_=wq.ap())
            wo_s = cp.tile([D, D], f32); nc.sync.dma_start(out=wo_s[:], in_=wo.ap())
            bkv_s = cp.tile([P, 2 * D], f32); nc.sync.dma_start(out=bkv_s[:], in_=bkv.ap())
            bq_s = cp.tile([P, D], f32); nc.sync.dma_start(out=bq_s[:], in_=bq.ap())
            iF3_s = cp.tile([P, 3 * P], f32); nc.sync.dma_start(out=iF3_s[:], in_=iotaF3.ap())
            iP_s = cp.tile([P, 1], f32); nc.sync.dma_start(out=iP_s[:], in_=iotaP.ap())
            on_s = cp.tile([1, P], f32); nc.sync.dma_start(out=on_s[:], in_=ones1.ap())
            zb_s = cp.tile([P, 1], f32); nc.vector.memset(zb_s[:], 0.0)

            kv_dram = dp.tile([n_pad, 2 * D], kv_dt)
            q_s = qp.tile([P, n_core], f32)

            for _rep in range(repeat):
                # ---- Phase A: kv = [x@Wk+bk | x@Wv+bv] for ALL nodes ----
                for b in range(n_all_blk if (_rep == 0 or rep_scope == 'all') else 0):
                    xt = xp.tile([P, P], f32, tag="xt")
                    nc.sync.dma_start(out=xt[:], in_=xT.ap()[:, b * P:(b + 1) * P])
                    pkv = pp.tile([P, 3 * P], f32, tag="mm")
                    nc.tensor.matmul(out=pkv[:, 0:D], lhsT=xt[:], rhs=wk_s[:],
                                     start=True, stop=True)
                    nc.tensor.matmul(out=pkv[:, D:2 * D], lhsT=xt[:], rhs=wv_s[:],
                                     start=True, stop=True)
                    kv_t = kp.tile([P, 2 * D], kv_dt, tag="kvw")
                    nc.vector.tensor_tensor(out=kv_t[:], in0=pkv[:, 0:2 * D], in1=bkv_s[:],
                                            op=mybir.AluOpType.add)
                    nc.sync.dma_start(out=kv_dram[b * P:(b + 1) * P, :], in_=kv_t[:])

                # ---- Phase B: q for this core's blocks, kept in SBUF ----
                for j in range(nblk_core if (_rep == 0 or rep_scope == 'all') else 0):
                    xt = xp.tile([P, P], f32, tag="xt")
                    nc.sync.dma_start(out=xt[:], in_=xTq.ap()[:, j * P:(j + 1) * P])
                    pq = pp.tile([P, 3 * P], f32, tag="mm")
                    nc.tensor.matmul(out=pq[:, 0:D], lhsT=xt[:], rhs=wq_s[:],
                                     start=True, stop=True)
                    nc.vector.tensor_tensor(out=q_s[:, j * P:(j + 1) * P], in0=pq[:, 0:D],
                                            in1=bq_s[:], op=mybir.AluOpType.add)

                # ---- Phase C: per-block edge processing ----
                inv_sqrt_dk = 1.0 / math.sqrt(D // 8)  # d_k = 16
                for j in range(nblk_core):
                    so_t = mp.tile([P, t_b], mybir.dt.int32, tag="so")
                    nc.sync.dma_start(out=so_t[:], in_=srcoff.ap()[:, j * t_b:(j + 1) * t_b])
                    oc_t = mp.tile([P, t_b], f32, tag="oc")
                    nc.sync.dma_start(out=oc_t[:], in_=offc.ap()[:, j * t_b:(j + 1) * t_b])
                    or_t = mp.tile([1, t_b * P], f32, tag="or")
                    nc.sync.dma_start(out=or_t[:], in_=offr.ap()[:, j * t_b * P:(j + 1) * t_b * P])

                    zT = pz.tile([P, P], f32, tag="zT")
                    nT = pz.tile([P, P], f32, tag="nT")
                    assert t_b % 3 == 0
                    ngrp = t_b // 3
                    st = {}

                    def s0(grp):
                        base = grp * 3
                        kv_g = gp.tile([P, 3 * 2 * D], kv_dt, tag="kv")
                        for i in range(3):
                            t = base + i
                            if seq_gather:
                                rr = ((j * t_b + t) * P) % (n_pad - P)
                                nc.sync.dma_start(out=kv_g[:, i * 2 * D:(i + 1) * 2 * D],
                                                  in_=kv_dram[rr:rr + P, :])
                            else:
                                nc.gpsimd.indirect_dma_start(
                                    out=kv_g[:, i * 2 * D:(i + 1) * 2 * D],
                                    out_offset=None, in_=kv_dram[:],
                                    in_offset=bass.IndirectOffsetOnAxis(
                                        ap=so_t[:, t:t + 1], axis=0))
                        if 'noS' in ablate:
                            s_sc = iF3_s
                        else:
                            s_sc = wp.tile([P, 3 * P], f32, tag="ssc")
                            nc.vector.tensor_tensor(
                                out=s_sc[:].rearrange("p (t c) -> p t c", t=3),
                                in0=iF3_s[:].rearrange("p (t c) -> p t c", t=3),
                                in1=oc_t[:, base:base + 3].to_broadcast([P, 3, P]),
                                op=mybir.AluOpType.is_equal)
                        offb = pp.tile([P, 3 * P], f32, tag="mm")
                        nc.tensor.matmul(out=offb[:], lhsT=on_s[:],
                                         rhs=or_t[:, base * P:(base + 3) * P],
                                         start=True, stop=True)
                        st[grp] = dict(kv_g=kv_g, s_sc=s_sc, offb=offb)

                    def s1(grp):
                        d = st[grp]
                        if 'noS' in ablate:
                            s_ga = iF3_s
                        else:
                            s_ga = wp.tile([P, 3 * P], f32, tag="sga")
                            nc.vector.tensor_tensor(out=s_ga[:], in0=d["offb"][:],
                                                    in1=iP_s[:].to_broadcast([P, 3 * P]),
                                                    op=mybir.AluOpType.is_equal)
                        qe = pp.tile([P, 3 * P], f32, tag="mm")
                        for i in range(3):
                            nc.tensor.matmul(out=qe[:, i * P:(i + 1) * P],
                                             lhsT=s_ga[:, i * P:(i + 1) * P],
                                             rhs=q_s[:, j * P:(j + 1) * P],
                                             start=True, stop=True)
                        kv3 = d["kv_g"][:].rearrange("p (t c) -> p t c", t=3)
                        t1 = wp.tile([P, 3 * D], f32, tag="t1")
                        nc.vector.tensor_tensor(
                            out=t1[:].rearrange("p (t c) -> p t c", t=3),
                            in0=qe[:].rearrange("p (t c) -> p t c", t=3),
                            in1=kv3[:, :, 0:D], op=mybir.AluOpType.mult)
                        m_t = wp.tile([P, 3 * D], f32, tag="m")
                        if 'noexp' in ablate:
                            nc.vector.tensor_copy(out=m_t[:], in_=t1[:])
                        else:
                            nc.scalar.activation(m_t[:], t1[:],
                                                 mybir.ActivationFunctionType.Exp,
                                                 bias=zb_s[:], scale=inv_sqrt_dk)
                        d.update(m_t=m_t)

                    def s2(grp):
                        d = st.pop(grp)
                        kv3 = d["kv_g"][:].rearrange("p (t c) -> p t c", t=3)
                        mv_t = wp.tile([P, 3 * D], f32, tag="mv")
                        nc.vector.tensor_tensor(
                            out=mv_t[:].rearrange("p (t c) -> p t c", t=3),
                            in0=d["m_t"][:].rearrange("p (t c) -> p t c", t=3),
                            in1=kv3[:, :, D:2 * D], op=mybir.AluOpType.mult)
                        if 'noacc' in ablate:
                            zx = pp.tile([P, 3 * P], f32, tag="mm")
                            for i in range(3):
                                nc.tensor.matmul(out=zx[:, i * P:(i + 1) * P],
                                                 lhsT=d["m_t"][:, i * P:(i + 1) * P],
                                                 rhs=d["s_sc"][:, i * P:(i + 1) * P],
                                                 start=True, stop=True)
                        else:
                            for i in range(3):
                                t = grp * 3 + i
                                nc.tensor.matmul(out=zT[:], lhsT=d["m_t"][:, i * P:(i + 1) * P],
                                                 rhs=d["s_sc"][:, i * P:(i + 1) * P],
                                                 start=(t == 0), stop=(t == t_b - 1))
                                nc.tensor.matmul(out=nT[:], lhsT=mv_t[:, i * P:(i + 1) * P],
                                                 rhs=d["s_sc"][:, i * P:(i + 1) * P],
                                                 start=(t == 0), stop=(t == t_b - 1))

                    for g in range(ngrp + 2):
                        if g < ngrp:
                            s0(g)
                        if 0 <= g - 1 < ngrp:
                            s1(g - 1)
                        if 0 <= g - 2 < ngrp:
                            s2(g - 2)
                    # epilogue: out_xT = nT / zT ; outT_blk = Wo.T-contract
                    rz = ep.tile([P, P], f32, tag="rz")
                    nc.vector.reciprocal(out=rz[:], in_=zT[:])
                    ox = ep.tile([P, P], f32, tag="ox")
                    nc.vector.tensor_tensor(out=ox[:], in0=nT[:], in1=rz[:],
                                            op=mybir.AluOpType.mult)
                    po = pp.tile([P, 3 * P], f32, tag="mm")
                    nc.tensor.matmul(out=po[:, 0:P], lhsT=wo_s[:], rhs=ox[:],
                                     start=True, stop=True)
                    o_sb = ep.tile([P, P], f32, tag="osb")
                    nc.vector.tensor_copy(out=o_sb[:], in_=po[:, 0:P])
                    nc.sync.dma_start(out=outT.ap()[:, j * P:(j + 1) * P], in_=o_sb[:])

    nc.compile()
    _cache[key] = nc
    return nc


def kernel(x, src, dst, Wq, bq, Wk, bk, Wv, bv, Wo, bo):
    x = np.asarray(x, dtype=np.float32)
    n, d = x.shape
    assert d == D
    e = src.shape[0]
    src = np.asarray(src, dtype=np.int64)
    dst = np.asarray(dst, dtype=np.int64)

    n_all_blk = math.ceil(n / P)
    # pad total blocks to a multiple of N_CORES
    n_all_blk = math.ceil(n_all_blk / N_CORES) * N_CORES
    n_pad = n_all_blk * P
    nblk_core = n_all_blk // N_CORES
    n_core = nblk_core * P

    # ---- host prep: sort edges by dst block ----
    order = np.argsort(dst, kind="stable")
    sdst = dst[order].astype(np.int64)
    ssrc = src[order].astype(np.int64)
    blk = (sdst // P).astype(np.int64)
    counts = np.bincount(blk, minlength=n_all_blk)
    starts = np.zeros(n_all_blk + 1, dtype=np.int64)
    np.cumsum(counts, out=starts[1:])
    t_b = max(1, int(math.ceil(counts.max() / P)))
    t_b = ((t_b + 2) // 3) * 3

    ncols = nblk_core * t_b
    srcoff_np = np.zeros((N_CORES, P, ncols), dtype=np.int32)
    offc_np = np.full((N_CORES, P, ncols), 255.0, dtype=np.float32)
    for b in range(n_all_blk):
        c, j = divmod(b, nblk_core)
        s0, s1 = starts[b], starts[b + 1]
        cnt = s1 - s0
        if cnt == 0:
            continue
        cols = np.arange(cnt) // P + j * t_b
        rows = np.arange(cnt) % P
        srcoff_np[c, rows, cols] = ssrc[s0:s1]
        offc_np[c, rows, cols] = (sdst[s0:s1] - b * P).astype(np.float32)
    # offr: same values, row-major per tile [1, ncols*P]
    offr_np = np.ascontiguousarray(
        offc_np.transpose(0, 2, 1).reshape(N_CORES, 1, ncols * P))

    x_pad = np.zeros((n_pad, D), dtype=np.float32)
    x_pad[:n] = x
    xT_np = np.ascontiguousarray(x_pad.T)

    iotaF3_np = np.tile(np.arange(P, dtype=np.float32)[None, :], (P, 3))
    iotaP_np = np.arange(P, dtype=np.float32)[:, None].copy()
    ones1_np = np.ones((1, P), dtype=np.float32)
    bkv_np = np.tile(np.concatenate([np.asarray(bk, np.float32),
                                     np.asarray(bv, np.float32)])[None, :], (P, 1))
    bq_np = np.tile(np.asarray(bq, np.float32)[None, :], (P, 1))

    nc = _build(nblk_core, t_b, n_all_blk)

    in_maps = []
    for c in range(N_CORES):
        in_maps.append({
            "xT": xT_np,
            "xTq": np.ascontiguousarray(xT_np[:, c * n_core:(c + 1) * n_core]),
            "wk": np.asarray(Wk, np.float32), "wv": np.asarray(Wv, np.float32),
            "wq": np.asarray(Wq, np.float32), "wo": np.asarray(Wo, np.float32),
            "bkv": bkv_np, "bq": bq_np,
            "iotaF3": iotaF3_np, "iotaP": iotaP_np, "ones1": ones1_np,
            "srcoff": srcoff_np[c], "offc": offc_np[c], "offr": offr_np[c],
        })
    results = bass2jax.run_bass_via_pjrt(nc, in_maps, n_cores=N_CORES)

    out = np.empty((n_pad, D), dtype=np.float32)
    for c in range(N_CORES):
        out[c * n_core:(c + 1) * n_core] = results[c]["outT"].T
    out = out[:n] + np.asarray(bo, np.float32)[None, :]
    return out.astype(np.float32)

